# revision 15
# baseline (speedup 1.0000x reference)
"""Trainium2 Bass kernel for nn_BASE_MAMBA_14018773254552.

Mamba block (d_model=128, d_inner=256, d_state=64, d_conv=4, L=1024, B=4)
+ input proj + classifier head.

Sharding: 8 cores = 4 batches x 2 d_inner-halves (128 channels each).
Each core computes its batch's full front-end (input proj, in_proj, conv,
x_proj) feature-major ([feature, time] tiles), then the selective scan for
its 128-channel half, and the partial out-proj + mean-pool. The host sums
the two channel-half partials per batch and runs the tiny classifier
(BatchNorm couples batches, so it cannot live on one core).

Scan pair layout: iteration over 64 channel pairs; partitions hold
(state n, channel j in pair). dt/u replication to the pair layout is done
with broadcast DMAs from a DRAM bounce (no selection-matrix matmuls); the
64-state reduction back to channels uses a tiny [128,2] parity matrix with
partition-offset matmul outputs.

Self-contained: hardcodes all shapes; builds + compiles the Bass program
once per process and runs it on cores 0-7 via run_bass_kernel_spmd.
"""
import numpy as np

try:
    import concourse.bacc as bacc
except ImportError:  # pragma: no cover - path fallback
    import sys
    for _p in ("/opt/trn_rl_repo", "/root/.axon_site/_ro/trn_rl_repo"):
        if _p not in sys.path:
            sys.path.insert(0, _p)
    import concourse.bacc as bacc

import ml_dtypes
import concourse.bass as bass
import concourse.mybir as mybir
import concourse.tile as tile
from concourse.bass_utils import run_bass_kernel_spmd

F32 = mybir.dt.float32
F32R = mybir.dt.float32r
BF16 = mybir.dt.bfloat16
AF = mybir.ActivationFunctionType
OP = mybir.AluOpType

B, L, CIN = 4, 1024, 20
DM, DS, DC = 128, 64, 4
DI = 256
DTR = 8
DH = 128          # channels per core (d_inner half)
NP = DH // 2      # 64 channel pairs in the scan
EPS = 1e-5

_cache = {}


def _build():
    nc = bacc.Bacc("TRN2", target_bir_lowering=False, debug=False, num_devices=8)

    # ---- I/O ----
    xt_d = nc.dram_tensor("xt", [CIN, L], F32R, kind="ExternalInput")
    wpT_d = nc.dram_tensor("wpT", [CIN, DM], F32R, kind="ExternalInput")
    bp_d = nc.dram_tensor("bp", [DM, 1], F32, kind="ExternalInput")
    wiT_d = nc.dram_tensor("wiT", [DM, 3 * DH], BF16, kind="ExternalInput")
    convw_d = nc.dram_tensor("convw", [DH, 2 * DC], F32, kind="ExternalInput")
    convb_d = nc.dram_tensor("convb", [DH, 2], F32, kind="ExternalInput")
    wxT_d = nc.dram_tensor("wxT", [DH, 2 * 136], BF16, kind="ExternalInput")
    wdtT_d = nc.dram_tensor("wdtT", [DTR, DH], F32R, kind="ExternalInput")
    bdt_d = nc.dram_tensor("bdt", [DH, 1], F32, kind="ExternalInput")
    aposn_d = nc.dram_tensor("aposn", [DH, DS], F32, kind="ExternalInput")
    dskip_d = nc.dram_tensor("dskip", [DH, 1], F32, kind="ExternalInput")
    woutT_d = nc.dram_tensor("woutT", [DH, DM], BF16, kind="ExternalInput")
    selE_d = nc.dram_tensor("selE", [DH, DS * DH], BF16, kind="ExternalInput")
    pooled_d = nc.dram_tensor("pooled", [DM, 1], F32, kind="ExternalOutput")
    dt_scr = nc.dram_tensor("dt_scr", [DH, L], BF16)
    u_scr = nc.dram_tensor("u_scr", [DH, L], BF16)
    bm_scr = nc.dram_tensor("bm_scr", [DS, L], BF16)
    cm_scr = nc.dram_tensor("cm_scr", [DS, L], BF16)

    HLF = (slice(0, 512), slice(512, 1024))

    with tile.TileContext(nc) as tc:
        with (
            tc.tile_pool(name="const", bufs=1) as cp,
            tc.tile_pool(name="work", bufs=1) as wp,
        ):
            # ---- load params ----
            xt = cp.tile([CIN, L], F32R)
            wpT = cp.tile([CIN, DM], F32R)
            bp = cp.tile([DM, 1], F32)
            wiT = cp.tile([DM, 3 * DH], BF16)
            convw = cp.tile([DH, 2 * DC], F32)
            convb = cp.tile([DH, 2], F32)
            wxT = cp.tile([DH, 2 * 136], BF16)
            wdtT = cp.tile([DTR, DH], F32R)
            bdt = cp.tile([DH, 1], F32)
            aposn = cp.tile([DH, DS], F32)
            dskip = cp.tile([DH, 1], F32)
            woutT = cp.tile([DH, DM], BF16)
            selE = cp.tile([DH, DS * DH], BF16)
            for t_, d_ in [(xt, xt_d), (wpT, wpT_d), (bp, bp_d), (wiT, wiT_d),
                           (convw, convw_d), (convb, convb_d), (wxT, wxT_d),
                           (wdtT, wdtT_d), (bdt, bdt_d), (aposn, aposn_d),
                           (dskip, dskip_d), (woutT, woutT_d)]:
                nc.sync.dma_start(t_[:], d_[:])
            nc.sync.dma_start(selE[:], selE_d[:])

            # ---- phase 1: front-end ----
            with tc.tile_pool(name="ps1", bufs=4, space="PSUM") as ps1:
                # h = Wp @ x + bp   [128 dm, 1024 t] in bf16
                h_ps = ps1.tile([DM, L], F32, tag="ps")
                for sl in HLF:
                    nc.tensor.matmul(h_ps[:, sl], wpT[:, :], xt[:, sl])
                h16 = wp.tile([DM, L], BF16)
                nc.vector.tensor_scalar(out=h16[:], in0=h_ps[:],
                                        scalar1=bp[:], scalar2=None,
                                        op0=OP.add)

                # xm_j = W_in[chunk_j] @ h   (j=0 own, j=1 other)
                xmp = []   # padded copies in SBUF
                for j in range(2):
                    xm_ps = ps1.tile([DH, L], F32, tag="ps")
                    for sl in HLF:
                        nc.tensor.matmul(
                            xm_ps[:, sl], wiT[:, j * DH:(j + 1) * DH], h16[:, sl])
                    pad = wp.tile([DH, DC - 1 + L], BF16, tag=f"xmp{j}")
                    nc.vector.memset(pad[:, 0:DC - 1], 0.0)
                    nc.vector.tensor_copy(pad[:, DC - 1:DC - 1 + L], xm_ps[:])
                    xmp.append(pad)

                z_ps = ps1.tile([DH, L], F32, tag="ps")
                for sl in HLF:
                    nc.tensor.matmul(z_ps[:, sl], wiT[:, 2 * DH:3 * DH],
                                     h16[:, sl])

                # causal depthwise conv + silu -> xc16_j (bf16)
                xc16 = []
                for j in range(2):
                    c01 = wp.tile([DH, L], BF16, tag=f"c01_{j}")
                    nc.vector.tensor_scalar(
                        out=c01[:], in0=xmp[j][:, 1:1 + L],
                        scalar1=convw[:, 4 * j + 1:4 * j + 2], scalar2=None,
                        op0=OP.mult)
                    nc.vector.scalar_tensor_tensor(
                        out=c01[:], in0=xmp[j][:, 0:L],
                        scalar=convw[:, 4 * j:4 * j + 1],
                        in1=c01[:], op0=OP.mult, op1=OP.add)
                    c23 = wp.tile([DH, L], BF16, tag=f"c23_{j}")
                    nc.vector.tensor_scalar(
                        out=c23[:], in0=xmp[j][:, 3:3 + L],
                        scalar1=convw[:, 4 * j + 3:4 * j + 4], scalar2=None,
                        op0=OP.mult)
                    nc.vector.scalar_tensor_tensor(
                        out=c23[:], in0=xmp[j][:, 2:2 + L],
                        scalar=convw[:, 4 * j + 2:4 * j + 3],
                        in1=c23[:], op0=OP.mult, op1=OP.add)
                    cacc = wp.tile([DH, L], BF16, tag=f"cacc{j}")
                    nc.vector.tensor_tensor(out=cacc[:], in0=c01[:],
                                            in1=c23[:], op=OP.add)
                    xc = wp.tile([DH, L], BF16, tag=f"xc{j}")
                    nc.scalar.activation(xc[:], cacc[:], AF.Silu,
                                         bias=convb[:, j:j + 1])
                    xc16.append(xc)

                # z-gate silu while its table set is loaded (tail-only use)
                zsig = wp.tile([DH, L], BF16)
                nc.scalar.activation(zsig[:], z_ps[:], AF.Silu)

                # dbc = W_x @ xc -> dtr [8,L] and [BmT;CmT] as one m=128
                dtr_ps = ps1.tile([DTR, L], F32, tag="ps")
                bc_ps = ps1.tile([2 * DS, L], F32, tag="ps")
                for (m0, msz, out_ps) in ((0, DTR, dtr_ps),
                                          (DTR, 2 * DS, bc_ps)):
                    for sl in HLF:
                        for j in range(2):
                            nc.tensor.matmul(
                                out_ps[:, sl],
                                wxT[:, 136 * j + m0:136 * j + m0 + msz],
                                xc16[j][:, sl],
                                start=(j == 0), stop=(j == 1))
                dtrT = wp.tile([DTR, L], F32R)
                nc.vector.tensor_copy(dtrT[:], dtr_ps[:])
                # dt = softplus(W_dt @ dtr + b_dt) = ln(1 + exp(raw + b_dt))
                # (uses only the natural_log_exp ACT table set)
                dt_ps = ps1.tile([DH, L], F32, tag="ps")
                for sl in HLF:
                    nc.tensor.matmul(dt_ps[:, sl], wdtT[:, :], dtrT[:, sl])
                bmT16 = wp.tile([DS, L], BF16)
                nc.vector.tensor_copy(bmT16[:], bc_ps[0:DS, :])
                cmT16 = wp.tile([DS, L], BF16)
                nc.vector.tensor_copy(cmT16[:], bc_ps[DS:2 * DS, :])
                nc.sync.dma_start(bm_scr[:], bmT16[:])
                nc.sync.dma_start(cm_scr[:], cmT16[:])
                spe = wp.tile([DH, L], F32)
                nc.scalar.activation(spe[:], dt_ps[:], AF.Exp, bias=bdt[:])
                DT = wp.tile([DH, L], BF16)  # holds +dt
                nc.scalar.activation(DT[:], spe[:], AF.Ln, bias=1.0)

            # U = dt * xc_own (bf16)
            U = wp.tile([DH, L], BF16)
            nc.vector.tensor_tensor(out=U[:], in0=DT[:], in1=xc16[0][:],
                                    op=OP.mult)

            # bounce dt/u to DRAM for pair-replication reads
            nc.sync.dma_start(dt_scr[:], DT[:])
            nc.sync.dma_start(u_scr[:], U[:])

            # Bm2x2/Cm2x2: [128, 2L] bf16 = 2x[rep], partition q = row q//2
            Bm2 = wp.tile([DH, 2 * L], BF16)
            Cm2 = wp.tile([DH, 2 * L], BF16)
            for scr, dst in ((bm_scr, Bm2), (cm_scr, Cm2)):
                sap = scr[:]
                for half in range(2):
                    nc.sync.dma_start(
                        dst[:, half * L:(half + 1) * L],
                        bass.AP(tensor=sap.tensor, offset=sap.offset,
                                ap=[sap.ap[0], [0, 2], sap.ap[1]]))

            # ---- phase 2: selective scan, pair layout (q = 2n + j) ----
            # iteration g covers pairs p0=2g, p0+1; pair p covers channels
            # d0=2p, d1=2p+1; partitions hold (n, j)
            with tc.tile_pool(name="psl", bufs=1, space="PSUM") as psl:
              with tc.tile_pool(name="sl", bufs=3) as slp:
                Y_ps = psl.tile([DH, L], F32, tag="Y")
                G4 = 2
                for g in range(NP // G4):
                    dtrep = slp.tile([DH, G4 * L], BF16, tag="dtrep")
                    sap = dt_scr[:]
                    for i in range(G4):
                        p = G4 * g + i
                        nc.scalar.dma_start(
                            dtrep[:, i * L:(i + 1) * L],
                            bass.AP(tensor=sap.tensor,
                                    offset=sap.offset + 2 * p * L,
                                    ap=[[0, DS], [L, 2], [1, L]]))
                    dAt = slp.tile([DH, G4 * L], F32, tag="dA")
                    for i in range(G4):
                        p = G4 * g + i
                        nc.scalar.activation(
                            dAt[:, i * L:(i + 1) * L],
                            dtrep[:, i * L:(i + 1) * L], AF.Exp,
                            scale=aposn[:, p:p + 1])
                    # zero the decay at pair boundaries so one long scan
                    # resets its state between pairs
                    for i in range(1, G4):
                        nc.vector.memset(dAt[:, i * L:i * L + 1], 0.0)
                    urep = slp.tile([DH, G4 * L], BF16, tag="urep")
                    sap = u_scr[:]
                    for i in range(G4):
                        p = G4 * g + i
                        nc.sync.dma_start(
                            urep[:, i * L:(i + 1) * L],
                            bass.AP(tensor=sap.tensor,
                                    offset=sap.offset + 2 * p * L,
                                    ap=[[0, DS], [L, 2], [1, L]]))
                    dBxt = slp.tile([DH, G4 * L], BF16, tag="dBx")
                    nc.vector.tensor_tensor(out=dBxt[:], in0=urep[:],
                                            in1=Bm2[:], op=OP.mult)
                    Ht = slp.tile([DH, G4 * L], BF16, tag="H")
                    nc.vector.tensor_tensor_scan(
                        out=Ht[:], data0=dAt[:], data1=dBxt[:],
                        initial=0.0, op0=OP.mult, op1=OP.add)
                    HCt = slp.tile([DH, G4 * L], BF16, tag="HC")
                    nc.vector.tensor_tensor(out=HCt[:], in0=Ht[:], in1=Cm2[:],
                                            op=OP.mult)
                    for i in range(G4):
                        p = G4 * g + i
                        selp = selE[:, DH * p:DH * (p + 1)]
                        for sl in HLF:
                            nc.tensor.matmul(
                                Y_ps[:, sl], selp,
                                HCt[:, i * L + sl.start:i * L + sl.stop],
                                start=(p == 0), stop=(p == NP - 1))

              # ---- tail: gate, out-proj, pool (pipelined by t-half) ----
              y2 = wp.tile([DH, L], BF16)
              y3 = wp.tile([DH, L], BF16)
              trash = wp.tile([DM, L], BF16)
              pooled_h = wp.tile([DM, 2], F32)
              pooled = wp.tile([DM, 1], F32)
              with tc.tile_pool(name="ps2", bufs=1, space="PSUM") as ps2:
                  out_ps = ps2.tile([DM, L], F32, tag="o")
                  for hi, sl in enumerate(HLF):
                      nc.vector.scalar_tensor_tensor(
                          out=y2[:, sl], in0=xc16[0][:, sl], scalar=dskip[:],
                          in1=Y_ps[:, sl], op0=OP.mult, op1=OP.add)
                      nc.vector.tensor_tensor(out=y3[:, sl], in0=y2[:, sl],
                                              in1=zsig[:, sl], op=OP.mult)
                      nc.tensor.matmul(out_ps[:, sl], woutT[:, :], y3[:, sl])
                      nc.scalar.activation(
                          trash[:, sl], out_ps[:, sl], AF.Identity,
                          scale=1.0 / L, accum_out=pooled_h[:, hi:hi + 1])
                  nc.vector.tensor_tensor(
                      out=pooled[:], in0=pooled_h[:, 0:1],
                      in1=pooled_h[:, 1:2], op=OP.add)
                  nc.scalar.dma_start(pooled_d[:], pooled[:])

    nc.compile()
    return nc


def _core_inputs(inputs, b, half):
    f32 = np.float32
    bf16 = ml_dtypes.bfloat16
    x = np.asarray(inputs["x"], f32)
    Wp = np.asarray(inputs["Wp"], f32)
    bp = np.asarray(inputs["bp"], f32)
    W_in = np.asarray(inputs["W_in"], f32)
    conv_w = np.asarray(inputs["conv_w"], f32)
    conv_b = np.asarray(inputs["conv_b"], f32)
    W_x = np.asarray(inputs["W_x"], f32)
    W_dt = np.asarray(inputs["W_dt"], f32)
    b_dt = np.asarray(inputs["b_dt"], f32)
    A_log = np.asarray(inputs["A_log"], f32)
    Dskip = np.asarray(inputs["Dskip"], f32)
    W_out = np.asarray(inputs["W_out"], f32)

    own = slice(half * DH, half * DH + DH)
    other = slice(DH, 2 * DH) if half == 0 else slice(0, DH)
    return {
        "xt": np.ascontiguousarray(x[b]),
        "wpT": np.ascontiguousarray(Wp.T),
        "bp": np.ascontiguousarray(bp[:, None]),
        "wiT": np.concatenate(
            [W_in[0:DI][own].T, W_in[0:DI][other].T,
             W_in[DI:2 * DI][own].T], axis=1).astype(bf16),
        "convw": np.concatenate([conv_w[own], conv_w[other]], axis=1),
        "convb": np.stack([conv_b[own], conv_b[other]], axis=1),
        "wxT": np.concatenate([W_x.T[own], W_x.T[other]], axis=1).astype(bf16),
        "wdtT": np.ascontiguousarray(W_dt[own].T),
        "bdt": np.ascontiguousarray(b_dt[own][:, None]),
        "aposn": -np.exp(_alog_pairs(A_log[own])),
        "dskip": np.ascontiguousarray(Dskip[own][:, None]),
        "woutT": np.ascontiguousarray(W_out[:, own].T).astype(bf16),
        "selE": _selE(),
    }


def _alog_pairs(alog_own):
    # alogp[q, p] = A_log[own][2p + q%2, q//2]
    out = np.empty((DH, DS), np.float32)
    q = np.arange(DH)
    for p in range(DS):
        out[:, p] = alog_own[2 * p + (q % 2), q // 2]
    return out


def _selE():
    if "selE" not in _cache:
        sel = np.zeros((DH, DS * DH), np.float32)
        q = np.arange(DH)
        for p in range(DS):
            sel[q, DH * p + 2 * p + (q % 2)] = 1.0
        _cache["selE"] = sel.astype(ml_dtypes.bfloat16)
    return _cache["selE"]


def kernel(**inputs) -> np.ndarray:
    if "nc" not in _cache:
        _cache["nc"] = _build()
    nc = _cache["nc"]

    in_maps = [_core_inputs(inputs, c // 2, c % 2) for c in range(8)]
    res = run_bass_kernel_spmd(nc, in_maps, core_ids=list(range(8)))

    pooled = np.zeros((B, DM), np.float32)
    for c in range(8):
        pooled[c // 2] += res.results[c]["pooled"][:, 0]

    # classifier head (host: BatchNorm couples all batches; ~300 flops)
    f32 = np.float32
    W1 = np.asarray(inputs["W1"], f32)
    b1 = np.asarray(inputs["b1"], f32)
    gamma = np.asarray(inputs["gamma"], f32)
    beta = np.asarray(inputs["beta"], f32)
    W2 = np.asarray(inputs["W2"], f32)
    b2 = np.asarray(inputs["b2"], f32)
    h1 = pooled @ W1.T + b1
    mu = h1.mean(axis=0)
    var = h1.var(axis=0)
    h1 = (h1 - mu) / np.sqrt(var + EPS) * gamma + beta
    h1 = np.maximum(h1, 0.0)
    return (h1 @ W2.T + b2).astype(np.float32)
